# revision 34
# baseline (speedup 1.0000x reference)
"""Mixtral decoder layer (GQA attention + top2-of-28-combination MoE) on 8 TRN2 cores.

SPMD design (one program; per-core behavior injected via inputs):
  - Attention head-sharded: core c owns q-heads {2c,2c+1} / kv-head c over ALL
    tokens (uniform causal loops). RoPE folded into an extra rotated-weight set
    (rope(q) = q*cos + (Rq)*sin). Scores transposed [s,t]; no max-subtract;
    softmax denominator via ones column appended to V.
  - Context re-sharded token-wise via AllToAll; O-proj + residual + rmsnorm2 +
    router + top2-of-28 routing per 256-token zigzag block {c, 15-c}.
  - MoE expert-parallel: AllGather x2 (bf16, natural) + routing rows; cumsum
    compaction -> indirect-DMA gather (capacity CAP); bf16 FFN (fp32 accum);
    weighted scatter into u-ordered [2048,1024] bf16 buffer; ReduceScatter;
    local residual add.
  - ln1/ln2 and 1/sqrt(HD) folded into weights host-side.
"""

import itertools

import numpy as np

import concourse.bass as bass
import concourse.tile as tile
from concourse import bacc, bass_utils, mybir

P = 128
B, S, H = 1, 2048, 1024
NH, KVH, HD = 16, 8, 64
E, TOPK, I = 8, 2, 3584
EPS = 1e-6
THETA = 1000000.0
NCORES = 8
NT = S // P
NPAIR = NT // 2
CAP = 640
NI = I // P
NCAP = CAP // P
BIGIDX = 3000.0
PADIDX = 4095
NEG = -1.0e30

f32 = mybir.dt.float32
f32r = mybir.dt.float32r
bf16 = mybir.dt.bfloat16
f8 = mybir.dt.float8e4
i32 = mybir.dt.int32
BF16_NP = mybir.dt.np(bf16)
F8_NP = mybir.dt.np(f8)
DR = mybir.MatmulPerfMode.DoubleRow
W13_SCALE = 32.0
W2_SCALE = 16.0
HP_SCALE = 16.0

COMBS = np.array(list(itertools.combinations(range(E), TOPK)), dtype=np.int64)

AluOp = mybir.AluOpType
Act = mybir.ActivationFunctionType
AxX = mybir.AxisListType.X


def _z_a_of_block(b):
    return (b, 0) if b < 8 else (15 - b, 1)


def _u_of_token():
    u = np.zeros(S, dtype=np.int64)
    for t_ in range(S):
        b = t_ // P
        z, a = _z_a_of_block(b)
        u[t_] = z * 256 + a * 128 + (t_ % P)
    return u


U_OF_T = _u_of_token()


def build_program(dbg: bool = False, n_unroll: int = 1, skip=()):
    """skip: subset of {"attn","ffn","coll","front"} for timing bisection."""
    nc = bacc.Bacc("TRN2", target_bir_lowering=False, debug=False,
                   num_devices=NCORES)

    def din(name, shape, dtype=f32):
        return nc.dram_tensor(name, list(shape), dtype, kind="ExternalInput").ap()

    t = {}
    t["hT_my"] = din("hT_my", [H, 256])
    t["hT_full"] = din("hT_full", [H, S])
    t["wqkvT"] = din("wqkvT", [H, 448])
    t["woT"] = din("woT", [H, H], bf16)
    t["gateT"] = din("gateT", [H, E])
    t["cosT"] = din("cosT", [P, S])
    t["sinT"] = din("sinT", [P, S])
    t["ident"] = din("ident", [P, P])
    t["tri"] = din("tri", [P, P])
    t["onescol"] = din("onescol", [P, 1])
    t["epscol"] = din("epscol", [P, 1])
    t["onescol_r"] = din("onescol_r", [P, 1], f32r)
    t["ones1_r"] = din("ones1_r", [1, P], f32r)
    t["onehot_r"] = din("onehot_r", [E, 1], f32r)
    t["mcomb_r"] = din("mcomb_r", [E, 28], f32r)
    t["selmat_r"] = din("selmat_r", [28, E], f32r)
    t["iota_t"] = din("iota_t", [P, NT], i32)
    t["padrow"] = din("padrow", [1, CAP], i32)
    t["zrow"] = din("zrow", [P, 4096], bf16)
    t["w13"] = din("w13", [NI, H, 256], f8)
    t["w2s"] = din("w2s", [NI, P, H], bf16)

    def dout(name, shape, dtype=f32):
        return nc.dram_tensor(name, list(shape), dtype, kind="ExternalOutput").ap()

    t["out_blk"] = dout("out_blk", [256, H])
    t["dbg"] = {}
    if dbg:
        for nm, shp, dt_ in [
            ("d_x1T", [H, 512], f32), ("d_qr", [P, S], f32), ("d_kr", [64, S], f32),
            ("d_ctxT", [P, S], f32), ("d_h2T", [H, 256], f32), ("d_rt", [16, 256], f32),
            ("d_idx", [P, NCAP], i32), ("d_xg", [P, CAP], f32), ("d_hp", [P, CAP], f32),
            ("d_moe", [256, H], f32),
        ]:
            t["dbg"][nm] = dout(nm, shp, dt_)

    rg = [list(range(NCORES))]
    with tile.TileContext(nc) as tc:
        for rep in range(n_unroll):
            _emit_once(nc, tc, rg, t, rep, skip)
    nc.compile()
    return nc


def _emit_once(nc, tc, rg, t, rep, skip=()):
    dbg = t["dbg"] if rep == 0 else {}
    r = f"r{rep}_"

    with nc.allow_low_precision(reason="f32r tiles share f32 bit layout"), \
         tc.tile_pool(name=r + "const", bufs=1) as cpool, \
         tc.tile_pool(name=r + "big", bufs=1) as big, \
         tc.tile_pool(name=r + "dram", bufs=1, space="DRAM") as dram:

        # ---- small constants (vector queue: keep SP free for x1 chunks) ----
        ident_sb = cpool.tile([P, P], f32)
        nc.scalar.dma_start(ident_sb[:], t["ident"])
        ident_r_sb = cpool.tile([P, P], f32r)
        nc.scalar.dma_start(ident_r_sb[:], t["ident"].bitcast(f32r))
        ident_b_sb = cpool.tile([P, P], bf16)
        nc.vector.tensor_copy(ident_b_sb[:], ident_sb[:])
        ident_f8_sb = cpool.tile([P, P], f8)
        nc.vector.tensor_copy(ident_f8_sb[:], ident_sb[:])
        tri_sb = cpool.tile([P, P], f32)
        nc.scalar.dma_start(tri_sb[:], t["tri"])
        onescol_sb = cpool.tile([P, 1], f32)
        nc.scalar.dma_start(onescol_sb[:], t["onescol"])
        epscol_sb = cpool.tile([P, 1], f32)
        nc.scalar.dma_start(epscol_sb[:], t["epscol"])
        onescol_r_sb = cpool.tile([P, 1], f32r)
        nc.scalar.dma_start(onescol_r_sb[:], t["onescol_r"])
        ones1_r_sb = cpool.tile([1, P], f32r)
        nc.scalar.dma_start(ones1_r_sb[:], t["ones1_r"])
        onehot_r_sb = cpool.tile([E, 1], f32r)
        nc.scalar.dma_start(onehot_r_sb[:], t["onehot_r"])
        mcomb_sb = cpool.tile([E, 28], f32r)
        nc.scalar.dma_start(mcomb_sb[:], t["mcomb_r"])
        selmat_sb = cpool.tile([28, E], f32r)
        nc.scalar.dma_start(selmat_sb[:], t["selmat_r"])
        iota_sb = cpool.tile([P, NT], i32)
        nc.scalar.dma_start(iota_sb[:], t["iota_t"])
        zrow_sb = cpool.tile([P, 4096], bf16)
        nc.scalar.dma_start(zrow_sb[:], t["zrow"])

        # ---- persistent activations ----
        hmy_sb = big.tile([P, 8, 256], f32)
        nc.scalar.dma_start(hmy_sb[:], t["hT_my"].rearrange("(kt p) n -> p kt n", p=P))
        h2_sb = big.tile([P, 8, 256], f32)
        fin_nat = big.tile([P, 2, H], f32)

        # ---- internal DRAM ----
        a2a_in = dram.tile([NCORES * P, 256], bf16)
        a2a_out = dram.tile([NCORES * P, 256], bf16)
        x2_in = dram.tile([256, H], f8)
        x2_all = dram.tile([S, H], f8, addr_space="Local" if "coll" in skip else "Shared")
        rt_in = dram.tile([16, 256], f32)
        rt_all = dram.tile([NCORES * 16, 256], f32, addr_space="Local" if "coll" in skip else "Shared")
        wm_nat = dram.tile([S, 1], f32)
        idx_dram = dram.tile([CAP, 1], i32)
        moe_acc = dram.tile([S, H], bf16)
        rs_out = dram.tile([256, H], bf16)

        # ================= attention scope =================
        with tc.tile_pool(name=r + "attn", bufs=1) as apool, \
             tc.tile_pool(name=r + "aw", bufs=2) as aw:

            wqkv_sb = apool.tile([P, 8, 448], f32r)
            nc.sync.dma_start(wqkv_sb[:],
                              t["wqkvT"].bitcast(f32r).rearrange("(kt p) n -> p kt n", p=P))
            x1_sb = apool.tile([P, 8, S], f32r)    # hT_full, normalized in place
            for xc in range(4):
                csl = slice(xc * 512, (xc + 1) * 512)
                nc.sync.dma_start(
                    x1_sb[:, :, csl],
                    t["hT_full"].bitcast(f32r)
                    .rearrange("(kt p) n -> p kt n", p=P)[:, :, csl])
            cos_sb = apool.tile([P, S], f32)
            nc.scalar.dma_start(cos_sb[:], t["cosT"])
            sin_sb = apool.tile([P, S], f32)
            nc.scalar.dma_start(sin_sb[:], t["sinT"])
            qr01_sb = apool.tile([64, 2, S], bf16)
            kr_sb = apool.tile([64, S], bf16)
            vb_sb = apool.tile([P, NT, 65], bf16)
            a2a_sb = apool.tile([P, NCORES, 256], bf16)

            nc.vector.tensor_copy(vb_sb[:, :, 64:65],
                                  onescol_sb[:].to_broadcast([P, NT, 1]))
            # rmsnorm1 scale rows (x1 left unnormalized; scale folded into
            # cos/sin and the V copy after projection)
            rmsps_cm = tc.tile_pool(name=r + "rmsps", bufs=1, space="PSUM"); rmsps = rmsps_cm.__enter__()
            qkvps_cm = tc.tile_pool(name=r + "qkvps", bufs=1, space="PSUM"); qkvps = qkvps_cm.__enter__()
            qkvps2_cm = tc.tile_pool(name=r + "qkvps2", bufs=2, space="PSUM"); qkvps2 = qkvps2_cm.__enter__()
            for ntile in range(4):
                nsl = slice(ntile * 512, (ntile + 1) * 512)
                # token scale sbc = 1/rms (computed from raw x in parallel with QKV)
                ssq = rmsps.tile([1, 512], f32, tag="ssq")
                for kt in range(8):
                    xsq = aw.tile([P, 512], f32r, tag="xsq")
                    nc.vector.tensor_mul(xsq[:], x1_sb[:, kt, nsl], x1_sb[:, kt, nsl])
                    nc.tensor.matmul(ssq[:], onescol_r_sb[:], xsq[:],
                                     start=(kt == 0), stop=(kt == 7))
                srow = aw.tile([1, 512], f32, tag="srow")
                nc.scalar.activation(srow[:], ssq[:], Act.Sqrt, bias=epscol_sb[0:1, :], scale=1.0 / H)
                srow_r = aw.tile([1, 512], f32r, tag="srow_r")
                nc.vector.reciprocal(srow_r[:], srow[:])
                sbc = rmsps.tile([P, 512], f32, tag="sbc")
                nc.tensor.matmul(sbc[:], ones1_r_sb[:1, :], srow_r[:],
                                 start=True, stop=True)
                # QKV on raw x; norm scale applied via cs/sn and the V copy
                q_ps = qkvps.tile([P, 512], f32, tag="q_ps")
                qR_ps = qkvps.tile([P, 512], f32, tag="qR_ps")
                kk_ps = qkvps.tile([P, 512], f32, tag="kk_ps")
                v_ps = qkvps.tile([64, 512], f32, tag="v_ps")
                for kt in range(8):
                    x1s = x1_sb[:, kt, nsl]
                    st, sp = kt == 0, kt == 7
                    nc.tensor.matmul(q_ps[:], wqkv_sb[:, kt, 0:128], x1s, start=st, stop=sp)
                    nc.tensor.matmul(qR_ps[:], wqkv_sb[:, kt, 128:256], x1s, start=st, stop=sp)
                    nc.tensor.matmul(kk_ps[:], wqkv_sb[:, kt, 256:384], x1s, start=st, stop=sp)
                    nc.tensor.matmul(v_ps[:], wqkv_sb[:, kt, 384:448], x1s, start=st, stop=sp)
                cs = aw.tile([P, 512], f32, tag="cs")
                sn = aw.tile([P, 512], f32, tag="sn")
                nc.vector.tensor_mul(cs[:], cos_sb[:, nsl], sbc[:])
                nc.vector.tensor_mul(sn[:], sin_sb[:, nsl], sbc[:])
                t1 = aw.tile([P, 512], f32, tag="rope1")
                t2 = aw.tile([P, 512], f32, tag="rope2")
                nc.vector.tensor_mul(t1[:], q_ps[:], cs[:])
                nc.vector.tensor_mul(t2[:], qR_ps[:], sn[:])
                nc.vector.tensor_add(qr01_sb[:, 0, nsl], t1[0:64, :], t2[0:64, :])
                nc.vector.tensor_add(qr01_sb[:, 1, nsl], t1[64:128, :], t2[64:128, :])
                nc.vector.tensor_mul(t1[:64, :], kk_ps[0:64, :], cs[0:64, :])
                nc.vector.tensor_mul(t2[:64, :], kk_ps[64:128, :], sn[0:64, :])
                nc.vector.tensor_add(kr_sb[:, nsl], t1[:64, :], t2[:64, :])
                v_f = aw.tile([64, 512], f32, tag="v_f")
                nc.vector.tensor_copy(v_f[:], v_ps[:])
                v_sb = aw.tile([64, 512], f32, tag="v_sb")
                nc.vector.tensor_mul(v_sb[:], v_f[:], sbc[0:64, :])
                for tt in range(4):
                    ti = ntile * 4 + tt
                    vtp = qkvps2.tile([P, 64], f32, tag="vtp")
                    nc.tensor.transpose(vtp[:], v_sb[:, tt * 128:(tt + 1) * 128],
                                        ident_sb[:64, :64])
                    nc.vector.tensor_copy(vb_sb[:, ti, 0:64], vtp[:])
            qkvps2_cm.__exit__(None, None, None)
            qkvps_cm.__exit__(None, None, None)
            rmsps_cm.__exit__(None, None, None)
            attps_cm = tc.tile_pool(name=r + "attps", bufs=2, space="PSUM"); attps = attps_cm.__enter__()
            attps1_cm = tc.tile_pool(name=r + "attps1", bufs=1, space="PSUM"); attps1 = attps1_cm.__enter__()
            attpsA_cm = tc.tile_pool(name=r + "attpsA", bufs=2, space="PSUM"); attpsA = attpsA_cm.__enter__()
            if dbg:
                nc.gpsimd.dma_start(dbg["d_qr"][0:64, :], qr01_sb[:, 0, :])
                nc.gpsimd.dma_start(dbg["d_qr"][64:128, :], qr01_sb[:, 1, :])
                nc.gpsimd.dma_start(dbg["d_kr"], kr_sb[:])

            # attention core (both q-heads fused in the free dim)
            for pr in ([] if "attn" in skip else range(NPAIR)):
                tcols = slice(pr * 256, (pr + 1) * 256)
                ctxA = attpsA.tile([65, 2, 128], f32, tag="ctxA")
                ctxB = attpsA.tile([65, 2, 128], f32, tag="ctxB")
                for si in range(2 * pr + 2):
                    full = si <= 2 * pr
                    stexp = aw.tile([P, 2, 256], bf16, tag="stexp")
                    if full:
                        st_ps = attps.tile([P, 2, 256], f32, tag="st_ps")
                        nc.tensor.matmul(st_ps[:], kr_sb[:, si * 128:(si + 1) * 128],
                                         qr01_sb[:, :, tcols], start=True, stop=True)
                        if si == 2 * pr:
                            for h in range(2):
                                nc.vector.tensor_add(st_ps[:, h, 0:128],
                                                     st_ps[:, h, 0:128], tri_sb[:])
                        nc.scalar.activation(stexp[:], st_ps[:], Act.Exp)
                        nc.tensor.matmul(ctxA[:], vb_sb[:, si, :], stexp[:, :, 0:128],
                                         start=(si == 0), stop=(si == 2 * pr))
                    else:
                        sth_ps = attps1.tile([P, 2, 128], f32, tag="sth_ps")
                        nc.tensor.matmul(sth_ps[:], kr_sb[:, si * 128:(si + 1) * 128],
                                         qr01_sb[:, :, pr * 256 + 128:(pr + 1) * 256],
                                         start=True, stop=True)
                        for h in range(2):
                            nc.vector.tensor_add(sth_ps[:, h, :],
                                                 sth_ps[:, h, :], tri_sb[:])
                        nc.scalar.activation(stexp[:, :, 128:256], sth_ps[:], Act.Exp)
                    nc.tensor.matmul(ctxB[:], vb_sb[:, si, :], stexp[:, :, 128:256],
                                     start=(si == 0), stop=(si == 2 * pr + 1))
                for half, ctx_ps in ((0, ctxA), (1, ctxB)):
                    rec = aw.tile([1, 2, 128], f32r, tag="rec")
                    nc.vector.reciprocal(rec[:], ctx_ps[64:65, :, :])
                    dbc = attps1.tile([64, 2, 128], f32, tag="dbc")
                    nc.tensor.matmul(dbc[:], ones1_r_sb[:1, 0:64], rec[:],
                                     start=True, stop=True)
                    dbc_sb = aw.tile([64, 2, 128], f32, tag="dbc_sb")
                    nc.scalar.activation(dbc_sb[:], dbc[:], Act.Copy)
                    ti = 2 * pr + half
                    z, a = _z_a_of_block(ti)
                    for h in range(2):
                        nc.vector.tensor_mul(
                            a2a_sb[h * 64:h * 64 + 64, z, a * 128:a * 128 + 128],
                            ctx_ps[0:64, h, :], dbc_sb[:, h, :])
            attpsA_cm.__exit__(None, None, None)
            attps1_cm.__exit__(None, None, None)
            attps_cm.__exit__(None, None, None)
            if "attn" in skip:
                nc.vector.tensor_copy(a2a_sb[:].rearrange("p c n -> p (c n)"),
                                      zrow_sb[:, 0:2048])
            nc.sync.dma_start(a2a_in[:].rearrange("(c p) n -> p c n", p=P), a2a_sb[:])
            if dbg:
                nc.gpsimd.dma_start(dbg["d_ctxT"],
                                    a2a_sb[:].rearrange("p c n -> p (c n)"))

        # zero moe_acc early (SP queue after input loads; overlaps attention)
        for z4 in range(4):
            nc.sync.dma_start(
                moe_acc[:].rearrange("(a p) n -> p a n", p=P)[:, z4 * 4:(z4 + 1) * 4, :],
                zrow_sb[:].rearrange("p (a n) -> p a n", a=4))

        if "coll" in skip:
            nc.sync.dma_start(a2a_out[:], a2a_in[:])
        else:
            nc.gpsimd.collective_compute(
                "AllToAll", AluOp.bypass, replica_groups=rg,
                ins=[a2a_in.opt()], outs=[a2a_out.opt()])

        # ================= O-proj + norm2 + router scope =================
        x2_sb = big.tile([P, 8, 256], f32r)
        with tc.tile_pool(name=r + "oproj", bufs=1) as opool, \
             tc.tile_pool(name=r + "ow", bufs=2) as ow:

            ctxmy_sb = opool.tile([P, 8, 256], bf16)
            nc.sync.dma_start(ctxmy_sb[:], a2a_out[:].rearrange("(c p) n -> p c n", p=P))
            wo_sb = opool.tile([P, 8, H], bf16)
            nc.sync.dma_start(wo_sb[:], t["woT"].rearrange("(kt p) n -> p kt n", p=P))
            o1_cm = tc.tile_pool(name=r + "o1", bufs=2, space="PSUM"); o1 = o1_cm.__enter__()
            for hd in range(8):
                o_ps = o1.tile([P, 256], f32, tag="o_ps")
                for dt_ in range(8):
                    nc.tensor.matmul(o_ps[:], wo_sb[:, dt_, hd * 128:(hd + 1) * 128],
                                     ctxmy_sb[:, dt_, :], start=(dt_ == 0), stop=(dt_ == 7))
                nc.vector.tensor_add(h2_sb[:, hd, :], o_ps[:], hmy_sb[:, hd, :])
            if dbg:
                nc.sync.dma_start(dbg["d_h2T"].rearrange("(kt p) n -> p kt n", p=P), h2_sb[:])

            # hoisted: h2 -> natural layout for the final residual add (fills
            # PE idle time during the x2 AllGather)
            for tt in range(2):
                for kt in range(8):
                    ht_ps = o1.tile([P, P], f32, tag="ht_ps")
                    nc.tensor.transpose(ht_ps[:], h2_sb[:, kt, tt * 128:(tt + 1) * 128],
                                        ident_sb[:])
                    nc.vector.tensor_copy(fin_nat[:, tt, kt * 128:(kt + 1) * 128],
                                          ht_ps[:])

            o1_cm.__exit__(None, None, None)
            # rmsnorm2
            o2_cm = tc.tile_pool(name=r + "o2", bufs=1, space="PSUM"); o2 = o2_cm.__enter__()
            ssq2 = o2.tile([1, 256], f32, tag="ssq2")
            for kt in range(8):
                xsq2 = ow.tile([P, 256], f32r, tag="xsq2")
                nc.scalar.activation(xsq2[:], h2_sb[:, kt, :], Act.Square)
                nc.tensor.matmul(ssq2[:], onescol_r_sb[:], xsq2[:],
                                 start=(kt == 0), stop=(kt == 7))
            srow2 = ow.tile([1, 256], f32, tag="srow2")
            nc.scalar.activation(srow2[:], ssq2[:], Act.Sqrt, bias=epscol_sb[0:1, :], scale=1.0 / H)
            srow2_r = ow.tile([1, 256], f32r, tag="srow2_r")
            nc.vector.reciprocal(srow2_r[:], srow2[:])
            sbc2 = o2.tile([P, 256], f32, tag="sbc2")
            nc.tensor.matmul(sbc2[:], ones1_r_sb[:1, :], srow2_r[:], start=True, stop=True)
            for kt in range(8):
                nc.vector.tensor_mul(x2_sb[:, kt, :], h2_sb[:, kt, :], sbc2[:])

            o2_cm.__exit__(None, None, None)
            o3_cm = tc.tile_pool(name=r + "o3", bufs=1, space="PSUM"); o3 = o3_cm.__enter__()
            # router + routing
            gate_sb = opool.tile([P, 8, E], f32r)
            nc.sync.dma_start(gate_sb[:],
                              t["gateT"].bitcast(f32r).rearrange("(kt p) n -> p kt n", p=P))
            rw_sb = ow.tile([P, 2, E], f32r, tag="rw")
            for tt in range(2):
                lg_ps = o3.tile([P, E], f32, tag="lg_ps")
                for kt in range(8):
                    nc.tensor.matmul(lg_ps[:], x2_sb[:, kt, tt * 128:(tt + 1) * 128],
                                     gate_sb[:, kt, :], start=(kt == 0), stop=(kt == 7))
                mx = ow.tile([P, 1], f32, tag="mx")
                nc.vector.tensor_reduce(mx[:], lg_ps[:], axis=AxX, op=AluOp.max)
                mxn = ow.tile([P, 1], f32, tag="mxn")
                nc.vector.tensor_scalar_mul(mxn[:], mx[:], -1.0)
                ex = ow.tile([P, E], f32, tag="ex")
                sm = ow.tile([P, 1], f32, tag="sm")
                nc.scalar.activation(ex[:], lg_ps[:], Act.Exp, bias=mxn[:], accum_out=sm[:])
                smr = ow.tile([P, 1], f32, tag="smr")
                nc.vector.reciprocal(smr[:], sm[:])
                nc.vector.tensor_scalar(rw_sb[:, tt, :], ex[:], smr[:], None, op0=AluOp.mult)
            rwT_sb = ow.tile([E, 256], f32r, tag="rwT")
            for tt in range(2):
                rwt_ps = o3.tile([E, P], f32r, tag="rwt_ps")
                nc.tensor.transpose(rwt_ps[:], rw_sb[:, tt, :], ident_r_sb[:])
                nc.vector.tensor_copy(rwT_sb[:, tt * 128:(tt + 1) * 128], rwt_ps[:])
            mask_sb = ow.tile([P, 2, 28], f32r, tag="mask")
            for tt in range(2):
                cb_ps = o3.tile([P, 28], f32, tag="cb_ps")
                nc.tensor.matmul(cb_ps[:], rwT_sb[:, tt * 128:(tt + 1) * 128],
                                 mcomb_sb[:], start=True, stop=True)
                mxc = ow.tile([P, 1], f32, tag="mxc")
                nc.vector.tensor_reduce(mxc[:], cb_ps[:], axis=AxX, op=AluOp.max)
                nc.vector.tensor_scalar(mask_sb[:, tt, :], cb_ps[:], mxc[:], None,
                                        op0=AluOp.is_ge)
            selT_ps = o3.tile([E, 256], f32, tag="selT_ps")
            for tt in range(2):
                mkt_ps = o3.tile([28, P], f32r, tag="mkt_ps")
                nc.tensor.transpose(mkt_ps[:], mask_sb[:, tt, :], ident_r_sb[:])
                mkt = ow.tile([28, P], f32r, tag="mkt")
                nc.vector.tensor_copy(mkt[:], mkt_ps[:])
                nc.tensor.matmul(selT_ps[:, tt * 128:(tt + 1) * 128], selmat_sb[:],
                                 mkt[:], start=True, stop=True)
            rwsel_sb = ow.tile([E, 256], f32r, tag="rwsel")
            nc.vector.tensor_mul(rwsel_sb[:], rwT_sb[:], selT_ps[:])
            nrm_ps = o3.tile([1, 256], f32, tag="nrm_ps")
            nc.tensor.matmul(nrm_ps[:], onescol_r_sb[:E, :], rwsel_sb[:],
                             start=True, stop=True)
            nrmr = ow.tile([1, 256], f32r, tag="nrmr")
            nc.vector.reciprocal(nrmr[:], nrm_ps[:])
            nbc_ps = o3.tile([E, 256], f32, tag="nbc_ps")
            nc.tensor.matmul(nbc_ps[:], ones1_r_sb[:1, :E], nrmr[:], start=True, stop=True)
            rts_sb = ow.tile([8, 256], f32, tag="rts_sb")
            nc.vector.tensor_copy(rts_sb[:], selT_ps[:])
            rtw_sb = ow.tile([8, 256], f32, tag="rtw_sb")
            nc.vector.tensor_mul(rtw_sb[:], rwsel_sb[:], nbc_ps[:])
            nc.sync.dma_start(rt_in[0:8, :], rts_sb[:])
            nc.sync.dma_start(rt_in[8:16, :], rtw_sb[:])
            if dbg:
                nc.sync.dma_start(dbg["d_rt"][0:8, :], rts_sb[:])
                nc.sync.dma_start(dbg["d_rt"][8:16, :], rtw_sb[:])

            o3_cm.__exit__(None, None, None)
            o4_cm = tc.tile_pool(name=r + "o4", bufs=2, space="PSUM"); o4 = o4_cm.__enter__()
            # x2 natural bf16
            x2n_sb = opool.tile([P, 2, H], f8)
            for tt in range(2):
                for kt in range(8):
                    xt_ps = o4.tile([P, P], f32r, tag="xt_ps")
                    nc.tensor.transpose(xt_ps[:], x2_sb[:, kt, tt * 128:(tt + 1) * 128],
                                        ident_r_sb[:])
                    nc.vector.tensor_copy(x2n_sb[:, tt, kt * 128:(kt + 1) * 128], xt_ps[:])
            nc.sync.dma_start(x2_in[:].rearrange("(a p) n -> p a n", p=P), x2n_sb[:])
            o4_cm.__exit__(None, None, None)

        if "coll" in skip:
            for cc_ in range(8):
                nc.sync.dma_start(x2_all[cc_ * 256:(cc_ + 1) * 256, :], x2_in[:])
                nc.sync.dma_start(rt_all[cc_ * 16:(cc_ + 1) * 16, :], rt_in[:])
        else:
            nc.gpsimd.collective_compute(
                "AllGather", AluOp.bypass, replica_groups=rg,
                ins=[rt_in.opt()], outs=[rt_all.opt()])
            nc.gpsimd.collective_compute(
                "AllGather", AluOp.bypass, replica_groups=rg,
                ins=[x2_in.opt()], outs=[x2_all.opt()])

        # ================= MoE scope =================
        with tc.tile_pool(name=r + "moe", bufs=1) as mpool, \
             tc.tile_pool(name=r + "mw", bufs=2) as mw:

            # routing rows -> pos/idx
            mi_cm = tc.tile_pool(name=r + "mi", bufs=1); mi = mi_cm.__enter__()
            sel8_3 = mi.tile([E, NCORES, 256], f32r, tag="selslot")
            nc.sync.dma_start(sel8_3[:],
                              rt_all[:].bitcast(f32r).rearrange("(r x) n -> x r n", x=16)[0:8, :, :])
            wm8_3 = mi.tile([E, NCORES, 256], f32r)
            nc.sync.dma_start(wm8_3[:],
                              rt_all[:].bitcast(f32r).rearrange("(r x) n -> x r n", x=16)[8:16, :, :])
            sel8 = sel8_3[:].rearrange("e r n -> e (r n)")
            wm8 = wm8_3[:].rearrange("e r n -> e (r n)")
            pos8 = mi.tile([E, S], f32, tag="posslot")
            nc.vector.tensor_tensor_scan(pos8[:], sel8, sel8, 0.0,
                                         op0=AluOp.add, op1=AluOp.bypass)
            nc.vector.tensor_scalar(pos8[:], pos8[:], -1.0 - BIGIDX, None, op0=AluOp.add)
            posm8 = mi.tile([E, S], f32r)
            nc.vector.tensor_mul(posm8[:], pos8[:], sel8)
            nc.vector.tensor_scalar(posm8[:], posm8[:], BIGIDX, None, op0=AluOp.add)
            m1_cm = tc.tile_pool(name=r + "m1", bufs=2, space="PSUM"); m1 = m1_cm.__enter__()
            posmy = mi.tile([1, S], f32, tag="posslot")
            wmmy = mi.tile([1, S], f32, tag="selslot")
            for ntile in range(4):
                nsl = slice(ntile * 512, (ntile + 1) * 512)
                pp = m1.tile([1, 512], f32, tag="pp")
                nc.tensor.matmul(pp[:], onehot_r_sb[:], posm8[:, nsl],
                                 start=True, stop=True)
                nc.vector.tensor_copy(posmy[:, nsl], pp[:])
                wp = m1.tile([1, 512], f32, tag="wp")
                nc.tensor.matmul(wp[:], onehot_r_sb[:], wm8[:, nsl],
                                 start=True, stop=True)
                nc.vector.tensor_copy(wmmy[:, nsl], wp[:])
            nc.sync.dma_start(wm_nat[:].rearrange("(a n) u -> a (n u)", a=1), wmmy[:])
            pad_sb = mw.tile([1, CAP], i32, tag="pad_sb")
            nc.sync.dma_start(pad_sb[:], t["padrow"])
            nc.sync.dma_start(idx_dram[:].rearrange("(a n) u -> a (n u)", a=1), pad_sb[:])
            for ti in ([] if "front" in skip else range(NT)):
                po_ps = m1.tile([P, 1], f32, tag="po_ps")
                nc.tensor.transpose(po_ps[:], posmy[:1, ti * 128:(ti + 1) * 128],
                                    ident_sb[:1, :1])
                po_i = mw.tile([P, 1], i32, tag="po_i")
                nc.vector.tensor_copy(po_i[:], po_ps[:])
                nc.gpsimd.indirect_dma_start(
                    out=idx_dram[:],
                    out_offset=bass.IndirectOffsetOnAxis(ap=po_i[:, :1], axis=0),
                    in_=iota_sb[:, ti:ti + 1], in_offset=None,
                    bounds_check=CAP - 1, oob_is_err=False)

            m1_cm.__exit__(None, None, None)
            mi_cm.__exit__(None, None, None)
            m2_cm = tc.tile_pool(name=r + "m2", bufs=2, space="PSUM"); m2 = m2_cm.__enter__()
            # gather + transpose
            xg_sb = mpool.tile([P, 8, CAP], f8)
            wmg_sb = mpool.tile([P, NCAP], f32)
            idx_tiles = []
            for j in range(NCAP):
                idxj = mpool.tile([P, 1], i32, tag=f"idxj{j}")
                idx_tiles.append(idxj)
                nc.sync.dma_start(idxj[:], idx_dram[j * P:(j + 1) * P, :])
                if "front" in skip:
                    continue
                gat = mw.tile([P, H], f8, tag="gat")
                nc.gpsimd.indirect_dma_start(
                    out=gat[:], out_offset=None, in_=x2_all[:],
                    in_offset=bass.IndirectOffsetOnAxis(ap=idxj[:, :1], axis=0),
                    bounds_check=S - 1, oob_is_err=False)
                nc.gpsimd.indirect_dma_start(
                    out=wmg_sb[:, j:j + 1], out_offset=None, in_=wm_nat[:],
                    in_offset=bass.IndirectOffsetOnAxis(ap=idxj[:, :1], axis=0),
                    bounds_check=S - 1, oob_is_err=False)
                for kt in range(8):
                    gt_ps = m2.tile([P, 2 * P], f8, tag="gt_ps")
                    nc.tensor.transpose(gt_ps[:, 0:256:2], gat[:, kt * 128:(kt + 1) * 128],
                                        ident_f8_sb[:])
                    nc.scalar.activation(xg_sb[:, kt, j * 128:(j + 1) * 128],
                                         gt_ps[:, 0:256:2], Act.Copy)
            if dbg:
                didx = mw.tile([P, NCAP], i32, tag="didx")
                for j in range(NCAP):
                    nc.vector.tensor_copy(didx[:, j:j + 1], idx_tiles[j][:])
                nc.sync.dma_start(dbg["d_idx"], didx[:])
                nc.gpsimd.dma_start(dbg["d_xg"], xg_sb[:, 0, :])

            m2_cm.__exit__(None, None, None)
            m3_cm = tc.tile_pool(name=r + "m3", bufs=4, space="PSUM"); m3 = m3_cm.__enter__()
            wpool_cm = tc.tile_pool(name=r + "wpre", bufs=6); wpre = wpool_cm.__enter__()
            # FFN phase A
            hp_sb = mpool.tile([P, NI, CAP], bf16)
            for it in ([] if "ffn" in skip else range(NI)):
                w13_sb = wpre.tile([P, 8, 256], f8, tag="w13_sb")
                nc.sync.dma_start(w13_sb[:], t["w13"][it].rearrange("(kt p) n -> p kt n", p=P))
                for hf in range(2):
                    csl = slice(hf * 320, hf * 320 + 320)
                    h1p = m3.tile([P, 320], f32, tag="h1")
                    h3p = m3.tile([P, 320], f32, tag="h3")
                    for kp in range(4):
                        ksl = slice(2 * kp, 2 * kp + 2)
                        st, sp = kp == 0, kp == 3
                        nc.tensor.matmul(h1p[:], w13_sb[:, ksl, 0:128], xg_sb[:, ksl, csl],
                                         start=st, stop=sp, perf_mode=DR)
                        nc.tensor.matmul(h3p[:], w13_sb[:, ksl, 128:256], xg_sb[:, ksl, csl],
                                         start=st, stop=sp, perf_mode=DR)
                    sg = mw.tile([P, 320], f32, tag="sg")
                    nc.scalar.activation(sg[:], h1p[:], Act.Sigmoid, scale=1.0 / W13_SCALE)
                    hp1 = mw.tile([P, 320], f32, tag="hp1")
                    nc.vector.tensor_mul(hp1[:], h1p[:], sg[:])
                    nc.vector.tensor_mul(hp_sb[:, it, csl], hp1[:], h3p[:])
            if dbg:
                nc.gpsimd.dma_start(dbg["d_hp"], hp_sb[:, 0, :])

            wpool_cm.__exit__(None, None, None)
            m3_cm.__exit__(None, None, None)
            # FFN phase B, computed transposed: out[tok, h] = sum_i hp[i,tok]*w2[i,h]
            # (hp chunks as weights, w2 rows streamed; no output transposes)
            out_nat = mpool.tile([P, NCAP, H], bf16)
            if "ffn" not in skip:
                m4_cm = tc.tile_pool(name=r + "m4", bufs=1, space="PSUM")
                m4 = m4_cm.__enter__()
                for hh in range(2):
                    hsl = slice(hh * 512, (hh + 1) * 512)
                    mo_ps = []
                    for j in range(NCAP):
                        mo_j = m4.tile([P, 512], f32, tag=f"mo{j}")
                        mo_ps.append(mo_j)
                    for it in range(NI):
                        w2t = mw.tile([P, 512], bf16, tag="w2t")
                        nc.sync.dma_start(w2t[:], t["w2s"][it][:, hsl])
                        for j in range(NCAP):
                            nc.tensor.matmul(mo_ps[j][:],
                                             hp_sb[:, it, j * 128:(j + 1) * 128],
                                             w2t[:], start=(it == 0), stop=(it == NI - 1))
                    for j in range(NCAP):
                        nc.vector.tensor_scalar(out_nat[:, j, hsl], mo_ps[j][:],
                                                wmg_sb[:, j:j + 1], None, op0=AluOp.mult)
                m4_cm.__exit__(None, None, None)
            for j in ([] if "ffn" in skip else range(NCAP)):
                nc.gpsimd.indirect_dma_start(
                    out=moe_acc[:],
                    out_offset=bass.IndirectOffsetOnAxis(ap=idx_tiles[j][:, :1], axis=0),
                    in_=out_nat[:, j, :], in_offset=None,
                    bounds_check=S - 1, oob_is_err=False)

        if "coll" in skip:
            nc.sync.dma_start(rs_out[:], moe_acc[0:256, :])
        else:
            nc.gpsimd.collective_compute(
                "ReduceScatter", AluOp.add, replica_groups=rg,
                ins=[moe_acc.opt()], outs=[rs_out.opt()])

        # ================= final =================
        with tc.tile_pool(name=r + "fin", bufs=2) as fw:
            rs_sb = fw.tile([P, 2, H], bf16, tag="rs_sb")
            nc.sync.dma_start(rs_sb[:], rs_out[:].rearrange("(a p) n -> p a n", p=P))
            if dbg:
                nc.gpsimd.dma_start(dbg["d_moe"].rearrange("(a p) n -> p a n", p=P),
                                    rs_sb[:])
            fin_sb = fw.tile([P, 2, H], f32, tag="fin_sb")
            for tt in range(2):
                nc.vector.tensor_add(fin_sb[:, tt, :], fin_nat[:, tt, :],
                                     rs_sb[:, tt, :])
            nc.sync.dma_start(t["out_blk"].rearrange("(a p) n -> p a n", p=P), fin_sb[:])


# ======================= host side =======================

def _rope_tables():
    pos = np.arange(S, dtype=np.float64)
    inv = 1.0 / (THETA ** (np.arange(0, HD, 2, dtype=np.float64) / HD))
    fr = pos[:, None] * inv[None, :]
    emb = np.concatenate([fr, fr], axis=-1)
    return np.cos(emb).astype(np.float32), np.sin(emb).astype(np.float32)


def _prep_inputs(inputs):
    hs = np.asarray(inputs["hidden_states"], np.float32)[0]
    ln1 = np.asarray(inputs["ln1_w"], np.float32)
    ln2 = np.asarray(inputs["ln2_w"], np.float32)
    wq = np.asarray(inputs["wq"], np.float32) * ln1[None, :] / np.sqrt(HD)
    wk = np.asarray(inputs["wk"], np.float32) * ln1[None, :]
    wv = np.asarray(inputs["wv"], np.float32) * ln1[None, :]
    wo = np.asarray(inputs["wo"], np.float32)
    gate = np.asarray(inputs["gate_w"], np.float32) * ln2[None, :]
    w1 = np.asarray(inputs["w1"], np.float32) * ln2[None, None, :]
    w3 = np.asarray(inputs["w3"], np.float32) * ln2[None, None, :]
    w2 = np.asarray(inputs["w2"], np.float32)

    cos, sin = _rope_tables()
    hT = np.ascontiguousarray(hs.T)

    def rot_w(w_head):  # [64, H] -> R @ w: rows = rotate_half structure
        return np.concatenate([-w_head[32:64], w_head[0:32]], axis=0)

    ident = np.eye(P, dtype=np.float32)
    sidx = np.arange(P)
    tri_m = np.where(sidx[:, None] <= sidx[None, :], 0.0, NEG).astype(np.float32)
    onescol = np.ones((P, 1), np.float32)
    EPS_ = EPS
    ones1 = np.ones((1, P), np.float32)
    mcomb = np.zeros((E, 28), np.float32)
    for ci, (a, b) in enumerate(COMBS):
        mcomb[a, ci] = 1.0
        mcomb[b, ci] = 1.0
    selmat = np.ascontiguousarray(mcomb.T)
    iota_t = (np.arange(NT)[None, :] * P + np.arange(P)[:, None]).astype(np.int32)
    padrow = np.full((1, CAP), PADIDX, np.int32)
    zrow = np.zeros((P, 4096), BF16_NP)
    cosT_d = np.ascontiguousarray(
        np.concatenate([cos.T, cos.T], axis=0))  # [128, S]
    sinT_d = np.ascontiguousarray(np.concatenate([sin.T, sin.T], axis=0))

    in_maps = []
    for c in range(NCORES):
        bA, bB = c, 15 - c
        tok = np.concatenate([np.arange(bA * P, bA * P + P),
                              np.arange(bB * P, bB * P + P)])
        qh0, qh1, kvh = 2 * c, 2 * c + 1, c
        wq0 = wq[qh0 * HD:(qh0 + 1) * HD]
        wq1 = wq[qh1 * HD:(qh1 + 1) * HD]
        wkc = wk[kvh * HD:(kvh + 1) * HD]
        wvc = wv[kvh * HD:(kvh + 1) * HD]
        wqkv = np.concatenate([
            wq0.T, wq1.T, rot_w(wq0).T, rot_w(wq1).T,
            wkc.T, rot_w(wkc).T, wvc.T], axis=1).astype(np.float32)
        onehot = np.zeros((E, 1), np.float32)
        onehot[c, 0] = 1.0
        m = {
            "hT_my": np.ascontiguousarray(hT[:, tok]),
            "hT_full": hT,
            "wqkvT": np.ascontiguousarray(wqkv),
            "woT": wo.T.astype(BF16_NP),
            "gateT": np.ascontiguousarray(gate.T),
            "cosT": cosT_d, "sinT": sinT_d,
            "ident": ident, "tri": tri_m,
            "onescol": onescol, "onescol_r": onescol, "ones1_r": ones1,
            "epscol": np.full((P, 1), EPS, np.float32),
            "onehot_r": onehot, "mcomb_r": mcomb, "selmat_r": selmat,
            "iota_t": iota_t, "padrow": padrow, "zrow": zrow,
            "w13": (np.ascontiguousarray(np.concatenate(
                [w1[c].reshape(NI, P, H).transpose(0, 2, 1),
                 w3[c].reshape(NI, P, H).transpose(0, 2, 1)],
                axis=2)) * W13_SCALE).astype(F8_NP),
            "w2s": (np.ascontiguousarray(w2[c].T.reshape(NI, P, H)) / (W13_SCALE * W13_SCALE)).astype(BF16_NP),
        }
        in_maps.append(m)
    return in_maps


_CACHE = {}


def _get_program(dbg=False, n_unroll=1):
    key = (dbg, n_unroll)
    if key not in _CACHE:
        _CACHE[key] = build_program(dbg=dbg, n_unroll=n_unroll)
    return _CACHE[key]


def run(inputs, dbg=False, n_unroll=1):
    nc = _get_program(dbg=dbg, n_unroll=n_unroll)
    in_maps = _prep_inputs(inputs)
    return bass_utils.run_bass_kernel_spmd(nc, in_maps, core_ids=list(range(NCORES)))


def assemble_output(res):
    out = np.zeros((S, H), np.float32)
    for c in range(NCORES):
        blk = res.results[c]["out_blk"]
        bA, bB = c, 15 - c
        out[bA * P:(bA + 1) * P] = blk[0:P]
        out[bB * P:(bB + 1) * P] = blk[P:256]
    return out.reshape(B, S, H)


def kernel(**inputs):
    res = run(inputs)
    return assemble_output(res)



# revision 51
# speedup vs baseline: 1.9650x; 1.9650x over previous
"""Mixtral decoder layer (GQA attention + top2-of-28-combination MoE) on 8 TRN2 cores.

SPMD design (one program; per-core behavior injected via inputs):
  - Attention head-sharded: core c owns q-heads {2c,2c+1} / kv-head c over ALL
    tokens (uniform causal loops). RoPE folded into an extra rotated-weight set
    (rope(q) = q*cos + (Rq)*sin). Scores transposed [s,t]; no max-subtract;
    softmax denominator via ones column appended to V.
  - Context re-sharded token-wise via AllToAll; O-proj + residual + rmsnorm2 +
    router + top2-of-28 routing per 256-token zigzag block {c, 15-c}.
  - MoE expert-parallel: AllGather x2 (bf16, natural) + routing rows; cumsum
    compaction -> indirect-DMA gather (capacity CAP); bf16 FFN (fp32 accum);
    weighted scatter into u-ordered [2048,1024] bf16 buffer; ReduceScatter;
    local residual add.
  - ln1/ln2 and 1/sqrt(HD) folded into weights host-side.
"""

import itertools

import numpy as np

import concourse.bass as bass
import concourse.tile as tile
from concourse import bacc, bass_utils, mybir

P = 128
B, S, H = 1, 2048, 1024
NH, KVH, HD = 16, 8, 64
E, TOPK, I = 8, 2, 3584
EPS = 1e-6
THETA = 1000000.0
NCORES = 8
NT = S // P
NPAIR = NT // 2
CAP = 640
NI = I // P
NCAP = CAP // P
BIGIDX = 3000.0
HUGE = 100000.0
SR = 272  # rows per source block in fused AG (256 tokens + 16 router rows)
PADIDX = 4095
NEG = -1.0e30

f32 = mybir.dt.float32
f32r = mybir.dt.float32r
bf16 = mybir.dt.bfloat16
f8 = mybir.dt.float8e4
i32 = mybir.dt.int32
BF16_NP = mybir.dt.np(bf16)
F8_NP = mybir.dt.np(f8)
DR = mybir.MatmulPerfMode.DoubleRow
W13_SCALE = 32.0
W2_SCALE = 16.0
HP_SCALE = 16.0

COMBS = np.array(list(itertools.combinations(range(E), TOPK)), dtype=np.int64)

AluOp = mybir.AluOpType
Act = mybir.ActivationFunctionType
AxX = mybir.AxisListType.X


def _z_a_of_block(b):
    return (b, 0) if b < 8 else (15 - b, 1)


def _u_of_token():
    u = np.zeros(S, dtype=np.int64)
    for t_ in range(S):
        b = t_ // P
        z, a = _z_a_of_block(b)
        u[t_] = z * 256 + a * 128 + (t_ % P)
    return u


U_OF_T = _u_of_token()


def build_program(dbg: bool = False, n_unroll: int = 1, skip=()):
    """skip: subset of {"attn","ffn","coll","front"} for timing bisection."""
    nc = bacc.Bacc("TRN2", target_bir_lowering=False, debug=False,
                   num_devices=NCORES)

    def din(name, shape, dtype=f32):
        return nc.dram_tensor(name, list(shape), dtype, kind="ExternalInput").ap()

    t = {}
    t["hT_my"] = din("hT_my", [H, 256])
    t["hT_full"] = din("hT_full", [H, S])
    t["wqkvT"] = din("wqkvT", [H, 448])
    t["woT"] = din("woT", [H, H], bf16)
    t["gateT"] = din("gateT", [H, E])
    t["cosT"] = din("cosT", [P, S])
    t["sinT"] = din("sinT", [P, S])
    t["ident"] = din("ident", [P, P])
    t["tri"] = din("tri", [P, P])
    t["onescol"] = din("onescol", [P, 1])
    t["epscol"] = din("epscol", [P, 1])
    t["onescol_r"] = din("onescol_r", [P, 1], f32r)
    t["ones1_r"] = din("ones1_r", [1, P], f32r)
    t["onehot_r"] = din("onehot_r", [E, 1], f32r)
    t["mcomb_r"] = din("mcomb_r", [E, 28], f32r)
    t["selmat_r"] = din("selmat_r", [28, E], f32r)
    t["iota3"] = din("iota3", [P, NT, 3])
    t["iota640"] = din("iota640", [P, CAP])
    t["zrow"] = din("zrow", [P, 4096], bf16)
    t["w13"] = din("w13", [NI, H, 256], f8)
    t["w2s"] = din("w2s", [NI, P, H], f8)

    def dout(name, shape, dtype=f32):
        return nc.dram_tensor(name, list(shape), dtype, kind="ExternalOutput").ap()

    t["out_blk"] = dout("out_blk", [256, H])
    t["dbg"] = {}
    if dbg:
        for nm, shp, dt_ in [
            ("d_x1T", [H, 512], f32), ("d_qr", [P, S], f32), ("d_kr", [64, S], f32),
            ("d_ctxT", [P, S], f32), ("d_h2T", [H, 256], f32), ("d_rt", [16, 256], f32),
            ("d_idx", [P, NCAP], i32), ("d_xg", [P, CAP], f32), ("d_hp", [P, CAP], f32),
            ("d_moe", [256, H], f32),
        ]:
            t["dbg"][nm] = dout(nm, shp, dt_)

    rg = [list(range(NCORES))]
    with tile.TileContext(nc) as tc:
        for rep in range(n_unroll):
            _emit_once(nc, tc, rg, t, rep, skip)
    nc.compile()
    return nc


def _emit_once(nc, tc, rg, t, rep, skip=()):
    dbg = t["dbg"] if rep == 0 else {}
    r = f"r{rep}_"

    with nc.allow_low_precision(reason="f32r tiles share f32 bit layout"), \
         tc.tile_pool(name=r + "const", bufs=1) as cpool, \
         tc.tile_pool(name=r + "big", bufs=1) as big, \
         tc.tile_pool(name=r + "dram", bufs=1, space="DRAM") as dram:

        # ---- small constants (vector queue: keep SP free for x1 chunks) ----
        ident_sb = cpool.tile([P, P], f32)
        nc.scalar.dma_start(ident_sb[:], t["ident"])
        ident_r_sb = cpool.tile([P, P], f32r)
        nc.scalar.dma_start(ident_r_sb[:], t["ident"].bitcast(f32r))
        ident_b_sb = cpool.tile([P, P], bf16)
        nc.vector.tensor_copy(ident_b_sb[:], ident_sb[:])
        ident_f8_sb = cpool.tile([P, P], f8)
        nc.vector.tensor_copy(ident_f8_sb[:], ident_sb[:])
        tri_sb = cpool.tile([P, P], f32)
        nc.scalar.dma_start(tri_sb[:], t["tri"])
        onescol_sb = cpool.tile([P, 1], f32)
        nc.scalar.dma_start(onescol_sb[:], t["onescol"])
        epscol_sb = cpool.tile([P, 1], f32)
        nc.scalar.dma_start(epscol_sb[:], t["epscol"])
        onescol_r_sb = cpool.tile([P, 1], f32r)
        nc.scalar.dma_start(onescol_r_sb[:], t["onescol_r"])
        ones1_r_sb = cpool.tile([1, P], f32r)
        nc.scalar.dma_start(ones1_r_sb[:], t["ones1_r"])
        onehot_r_sb = cpool.tile([E, 1], f32r)
        nc.scalar.dma_start(onehot_r_sb[:], t["onehot_r"])
        mcomb_sb = cpool.tile([E, 28], f32r)
        nc.scalar.dma_start(mcomb_sb[:], t["mcomb_r"])
        selmat_sb = cpool.tile([28, E], f32r)
        nc.scalar.dma_start(selmat_sb[:], t["selmat_r"])
        iota3_sb = cpool.tile([P, NT, 3], f32r)
        nc.scalar.dma_start(iota3_sb[:], t["iota3"].bitcast(f32r))
        iota640_sb = cpool.tile([P, CAP], f32)
        nc.scalar.dma_start(iota640_sb[:], t["iota640"])
        zrow_sb = cpool.tile([P, 4096], bf16)
        nc.scalar.dma_start(zrow_sb[:], t["zrow"])

        # ---- persistent activations ----
        hmy_sb = big.tile([P, 8, 256], f32)
        nc.scalar.dma_start(hmy_sb[:], t["hT_my"].rearrange("(kt p) n -> p kt n", p=P))
        h2_sb = big.tile([P, 8, 256], f32)
        fin_nat = big.tile([P, 2, H], f32)

        # ---- internal DRAM ----
        a2a_in = dram.tile([NCORES * P, 256], bf16)
        a2a_out = dram.tile([NCORES * P, 256], bf16)
        xa_in = dram.tile([256, H], f8)
        xa_all = dram.tile([S, H], f8,
                           addr_space="Local" if "coll" in skip else "Shared")
        rt_in = dram.tile([16, 256], f32)
        rt_all = dram.tile([NCORES * 16, 256], f32,
                           addr_space="Local" if "coll" in skip else "Shared")
        moe_acc = dram.tile([S, H], bf16)
        rs_out = dram.tile([256, H], bf16)

        # ================= attention scope =================
        with tc.tile_pool(name=r + "attn", bufs=1) as apool, \
             tc.tile_pool(name=r + "aw", bufs=2) as aw:

            wqkv_sb = apool.tile([P, 8, 448], f32r)
            nc.sync.dma_start(wqkv_sb[:],
                              t["wqkvT"].bitcast(f32r).rearrange("(kt p) n -> p kt n", p=P))
            x1_sb = apool.tile([P, 8, S], f32r)    # hT_full, normalized in place
            for xc in range(4):
                csl = slice(xc * 512, (xc + 1) * 512)
                nc.sync.dma_start(
                    x1_sb[:, :, csl],
                    t["hT_full"].bitcast(f32r)
                    .rearrange("(kt p) n -> p kt n", p=P)[:, :, csl])
            cos_sb = apool.tile([P, S], f32)
            nc.scalar.dma_start(cos_sb[:], t["cosT"])
            sin_sb = apool.tile([P, S], f32)
            nc.scalar.dma_start(sin_sb[:], t["sinT"])
            qr01_sb = apool.tile([64, 2, S], bf16)
            kr_sb = apool.tile([64, S], bf16)
            vb_sb = apool.tile([P, NT, 65], bf16)
            a2a_sb = apool.tile([P, NCORES, 256], bf16)

            nc.vector.tensor_copy(vb_sb[:, :, 64:65],
                                  onescol_sb[:].to_broadcast([P, NT, 1]))
            # rmsnorm1 scale rows (x1 left unnormalized; scale folded into
            # cos/sin and the V copy after projection)
            rmsps_cm = tc.tile_pool(name=r + "rmsps", bufs=1, space="PSUM"); rmsps = rmsps_cm.__enter__()
            qkvps_cm = tc.tile_pool(name=r + "qkvps", bufs=1, space="PSUM"); qkvps = qkvps_cm.__enter__()
            qkvps2_cm = tc.tile_pool(name=r + "qkvps2", bufs=2, space="PSUM"); qkvps2 = qkvps2_cm.__enter__()
            for ntile in range(4):
                nsl = slice(ntile * 512, (ntile + 1) * 512)
                # token scale sbc = 1/rms (computed from raw x in parallel with QKV)
                ssq = rmsps.tile([1, 512], f32, tag="ssq")
                for kt in range(8):
                    xsq = aw.tile([P, 512], f32r, tag="xsq")
                    nc.gpsimd.tensor_mul(xsq[:], x1_sb[:, kt, nsl], x1_sb[:, kt, nsl])
                    nc.tensor.matmul(ssq[:], onescol_r_sb[:], xsq[:],
                                     start=(kt == 0), stop=(kt == 7))
                srow = aw.tile([1, 512], f32, tag="srow")
                nc.scalar.activation(srow[:], ssq[:], Act.Sqrt, bias=epscol_sb[0:1, :], scale=1.0 / H)
                srow_r = aw.tile([1, 512], f32r, tag="srow_r")
                nc.vector.reciprocal(srow_r[:], srow[:])
                sbc = rmsps.tile([P, 512], f32, tag="sbc")
                nc.tensor.matmul(sbc[:], ones1_r_sb[:1, :], srow_r[:],
                                 start=True, stop=True)
                # QKV on raw x; norm scale applied via cs/sn and the V copy
                q_ps = qkvps.tile([P, 512], f32, tag="q_ps")
                qR_ps = qkvps.tile([P, 512], f32, tag="qR_ps")
                kk_ps = qkvps.tile([P, 512], f32, tag="kk_ps")
                v_ps = qkvps.tile([64, 512], f32, tag="v_ps")
                for kt in range(8):
                    x1s = x1_sb[:, kt, nsl]
                    st, sp = kt == 0, kt == 7
                    nc.tensor.matmul(q_ps[:], wqkv_sb[:, kt, 0:128], x1s, start=st, stop=sp)
                    nc.tensor.matmul(qR_ps[:], wqkv_sb[:, kt, 128:256], x1s, start=st, stop=sp)
                    nc.tensor.matmul(kk_ps[:], wqkv_sb[:, kt, 256:384], x1s, start=st, stop=sp)
                    nc.tensor.matmul(v_ps[:], wqkv_sb[:, kt, 384:448], x1s, start=st, stop=sp)
                sbs = aw.tile([P, 512], f32, tag="sbs")
                nc.vector.tensor_copy(sbs[:], sbc[:])
                cs = aw.tile([P, 512], f32, tag="cs")
                sn = aw.tile([P, 512], f32, tag="sn")
                nc.gpsimd.tensor_mul(cs[:], cos_sb[:, nsl], sbs[:])
                nc.gpsimd.tensor_mul(sn[:], sin_sb[:, nsl], sbs[:])
                t1 = aw.tile([P, 512], f32, tag="rope1")
                t2 = aw.tile([P, 512], f32, tag="rope2")
                nc.vector.tensor_mul(t1[:], q_ps[:], cs[:])
                nc.vector.tensor_mul(t2[:], qR_ps[:], sn[:])
                nc.vector.tensor_add(qr01_sb[:, 0, nsl], t1[0:64, :], t2[0:64, :])
                nc.vector.tensor_add(qr01_sb[:, 1, nsl], t1[64:128, :], t2[64:128, :])
                nc.vector.tensor_mul(t1[:64, :], kk_ps[0:64, :], cs[0:64, :])
                nc.vector.tensor_mul(t2[:64, :], kk_ps[64:128, :], sn[0:64, :])
                nc.vector.tensor_add(kr_sb[:, nsl], t1[:64, :], t2[:64, :])
                v_f = aw.tile([64, 512], f32, tag="v_f")
                nc.vector.tensor_copy(v_f[:], v_ps[:])
                v_sb = aw.tile([64, 512], f32, tag="v_sb")
                nc.gpsimd.tensor_mul(v_sb[:], v_f[:], sbs[0:64, :])
                for tt in range(4):
                    ti = ntile * 4 + tt
                    vtp = qkvps2.tile([P, 64], f32, tag="vtp")
                    nc.tensor.transpose(vtp[:], v_sb[:, tt * 128:(tt + 1) * 128],
                                        ident_sb[:64, :64])
                    nc.vector.tensor_copy(vb_sb[:, ti, 0:64], vtp[:])
            qkvps2_cm.__exit__(None, None, None)
            qkvps_cm.__exit__(None, None, None)
            rmsps_cm.__exit__(None, None, None)
            attps_cm = tc.tile_pool(name=r + "attps", bufs=2, space="PSUM"); attps = attps_cm.__enter__()
            attps1_cm = tc.tile_pool(name=r + "attps1", bufs=1, space="PSUM"); attps1 = attps1_cm.__enter__()
            attpsA_cm = tc.tile_pool(name=r + "attpsA", bufs=2, space="PSUM"); attpsA = attpsA_cm.__enter__()
            if dbg:
                nc.gpsimd.dma_start(dbg["d_qr"][0:64, :], qr01_sb[:, 0, :])
                nc.gpsimd.dma_start(dbg["d_qr"][64:128, :], qr01_sb[:, 1, :])
                nc.gpsimd.dma_start(dbg["d_kr"], kr_sb[:])

            # attention core (both q-heads fused in the free dim)
            for pr in ([] if "attn" in skip else range(NPAIR)):
                tcols = slice(pr * 256, (pr + 1) * 256)
                ctxA = attpsA.tile([65, 2, 128], f32, tag="ctxA")
                ctxB = attpsA.tile([65, 2, 128], f32, tag="ctxB")
                for si in range(2 * pr + 2):
                    full = si <= 2 * pr
                    stexp = aw.tile([P, 2, 256], bf16, tag="stexp")
                    if full:
                        st_ps = attps.tile([P, 2, 256], f32, tag="st_ps")
                        nc.tensor.matmul(st_ps[:], kr_sb[:, si * 128:(si + 1) * 128],
                                         qr01_sb[:, :, tcols], start=True, stop=True)
                        if si == 2 * pr:
                            for h in range(2):
                                nc.vector.tensor_add(st_ps[:, h, 0:128],
                                                     st_ps[:, h, 0:128], tri_sb[:])
                        nc.scalar.activation(stexp[:], st_ps[:], Act.Exp)
                        nc.tensor.matmul(ctxA[:], vb_sb[:, si, :], stexp[:, :, 0:128],
                                         start=(si == 0), stop=(si == 2 * pr))
                    else:
                        sth_ps = attps1.tile([P, 2, 128], f32, tag="sth_ps")
                        nc.tensor.matmul(sth_ps[:], kr_sb[:, si * 128:(si + 1) * 128],
                                         qr01_sb[:, :, pr * 256 + 128:(pr + 1) * 256],
                                         start=True, stop=True)
                        for h in range(2):
                            nc.vector.tensor_add(sth_ps[:, h, :],
                                                 sth_ps[:, h, :], tri_sb[:])
                        nc.scalar.activation(stexp[:, :, 128:256], sth_ps[:], Act.Exp)
                    nc.tensor.matmul(ctxB[:], vb_sb[:, si, :], stexp[:, :, 128:256],
                                     start=(si == 0), stop=(si == 2 * pr + 1))
                for half, ctx_ps in ((0, ctxA), (1, ctxB)):
                    rec = aw.tile([1, 2, 128], f32r, tag="rec")
                    nc.vector.reciprocal(rec[:], ctx_ps[64:65, :, :])
                    dbc = attps1.tile([64, 2, 128], f32, tag="dbc")
                    nc.tensor.matmul(dbc[:], ones1_r_sb[:1, 0:64], rec[:],
                                     start=True, stop=True)
                    dbc_sb = aw.tile([64, 2, 128], f32, tag="dbc_sb")
                    nc.vector.tensor_copy(dbc_sb[:], dbc[:])
                    ti = 2 * pr + half
                    z, a = _z_a_of_block(ti)
                    for h in range(2):
                        nc.vector.tensor_mul(
                            a2a_sb[h * 64:h * 64 + 64, z, a * 128:a * 128 + 128],
                            ctx_ps[0:64, h, :], dbc_sb[:, h, :])
            attpsA_cm.__exit__(None, None, None)
            attps1_cm.__exit__(None, None, None)
            attps_cm.__exit__(None, None, None)
            if "attn" in skip:
                nc.vector.tensor_copy(a2a_sb[:].rearrange("p c n -> p (c n)"),
                                      zrow_sb[:, 0:2048])
            nc.sync.dma_start(a2a_in[:].rearrange("(c p) n -> p c n", p=P), a2a_sb[:])
            if dbg:
                nc.gpsimd.dma_start(dbg["d_ctxT"],
                                    a2a_sb[:].rearrange("p c n -> p (c n)"))

        # zero moe_acc early (SP queue after input loads; overlaps attention)
        for z4 in range(4):
            nc.sync.dma_start(
                moe_acc[:].rearrange("(a p) n -> p a n", p=P)[:, z4 * 4:(z4 + 1) * 4, :],
                zrow_sb[:].rearrange("p (a n) -> p a n", a=4))

        if "coll" in skip:
            nc.sync.dma_start(a2a_out[:], a2a_in[:])
        else:
            nc.gpsimd.collective_compute(
                "AllToAll", AluOp.bypass, replica_groups=rg,
                ins=[a2a_in.opt()], outs=[a2a_out.opt()])

        # ================= O-proj + norm2 + router scope =================
        x2_sb = big.tile([P, 8, 256], f32r)
        with tc.tile_pool(name=r + "oproj", bufs=1) as opool, \
             tc.tile_pool(name=r + "ow", bufs=2) as ow:

            ctxmy_sb = opool.tile([P, 8, 256], bf16)
            nc.sync.dma_start(ctxmy_sb[:], a2a_out[:].rearrange("(c p) n -> p c n", p=P))
            wo_sb = opool.tile([P, 8, H], bf16)
            nc.sync.dma_start(wo_sb[:], t["woT"].rearrange("(kt p) n -> p kt n", p=P))
            o1_cm = tc.tile_pool(name=r + "o1", bufs=2, space="PSUM"); o1 = o1_cm.__enter__()
            for hd in range(8):
                o_ps = o1.tile([P, 256], f32, tag="o_ps")
                for dt_ in range(8):
                    nc.tensor.matmul(o_ps[:], wo_sb[:, dt_, hd * 128:(hd + 1) * 128],
                                     ctxmy_sb[:, dt_, :], start=(dt_ == 0), stop=(dt_ == 7))
                nc.vector.tensor_add(h2_sb[:, hd, :], o_ps[:], hmy_sb[:, hd, :])
            if dbg:
                nc.sync.dma_start(dbg["d_h2T"].rearrange("(kt p) n -> p kt n", p=P), h2_sb[:])

            o1_cm.__exit__(None, None, None)
            # rmsnorm2
            o2_cm = tc.tile_pool(name=r + "o2", bufs=1, space="PSUM"); o2 = o2_cm.__enter__()
            ssq2 = o2.tile([1, 256], f32, tag="ssq2")
            for kt in range(8):
                xsq2 = ow.tile([P, 256], f32r, tag="xsq2")
                nc.gpsimd.tensor_mul(xsq2[:], h2_sb[:, kt, :], h2_sb[:, kt, :])
                nc.tensor.matmul(ssq2[:], onescol_r_sb[:], xsq2[:],
                                 start=(kt == 0), stop=(kt == 7))
            srow2 = ow.tile([1, 256], f32, tag="srow2")
            nc.scalar.activation(srow2[:], ssq2[:], Act.Sqrt, bias=epscol_sb[0:1, :], scale=1.0 / H)
            srow2_r = ow.tile([1, 256], f32r, tag="srow2_r")
            nc.vector.reciprocal(srow2_r[:], srow2[:])
            sbc2 = o2.tile([P, 256], f32, tag="sbc2")
            nc.tensor.matmul(sbc2[:], ones1_r_sb[:1, :], srow2_r[:], start=True, stop=True)
            for kt in range(8):
                nc.vector.tensor_mul(x2_sb[:, kt, :], h2_sb[:, kt, :], sbc2[:])

            o2_cm.__exit__(None, None, None)
            o3_cm = tc.tile_pool(name=r + "o3", bufs=1, space="PSUM"); o3 = o3_cm.__enter__()
            # router + routing
            gate_sb = opool.tile([P, 8, E], f32r)
            nc.sync.dma_start(gate_sb[:],
                              t["gateT"].bitcast(f32r).rearrange("(kt p) n -> p kt n", p=P))
            rw_sb = ow.tile([P, 2, E], f32r, tag="rw")
            for tt in range(2):
                lg_ps = o3.tile([P, E], f32, tag="lg_ps")
                for kt in range(8):
                    nc.tensor.matmul(lg_ps[:], x2_sb[:, kt, tt * 128:(tt + 1) * 128],
                                     gate_sb[:, kt, :], start=(kt == 0), stop=(kt == 7))
                mx = ow.tile([P, 1], f32, tag="mx")
                nc.vector.tensor_reduce(mx[:], lg_ps[:], axis=AxX, op=AluOp.max)
                mxn = ow.tile([P, 1], f32, tag="mxn")
                nc.vector.tensor_scalar_mul(mxn[:], mx[:], -1.0)
                ex = ow.tile([P, E], f32, tag="ex")
                sm = ow.tile([P, 1], f32, tag="sm")
                nc.scalar.activation(ex[:], lg_ps[:], Act.Exp, bias=mxn[:], accum_out=sm[:])
                smr = ow.tile([P, 1], f32, tag="smr")
                nc.vector.reciprocal(smr[:], sm[:])
                nc.vector.tensor_scalar(rw_sb[:, tt, :], ex[:], smr[:], None, op0=AluOp.mult)
            rwT_sb = ow.tile([E, 256], f32r, tag="rwT")
            for tt in range(2):
                rwt_ps = o3.tile([E, P], f32r, tag="rwt_ps")
                nc.tensor.transpose(rwt_ps[:], rw_sb[:, tt, :], ident_r_sb[:])
                nc.vector.tensor_copy(rwT_sb[:, tt * 128:(tt + 1) * 128], rwt_ps[:])
            mask_sb = ow.tile([P, 2, 28], f32r, tag="mask")
            for tt in range(2):
                cb_ps = o3.tile([P, 28], f32, tag="cb_ps")
                nc.tensor.matmul(cb_ps[:], rwT_sb[:, tt * 128:(tt + 1) * 128],
                                 mcomb_sb[:], start=True, stop=True)
                mxc = ow.tile([P, 1], f32, tag="mxc")
                nc.vector.tensor_reduce(mxc[:], cb_ps[:], axis=AxX, op=AluOp.max)
                nc.vector.tensor_scalar(mask_sb[:, tt, :], cb_ps[:], mxc[:], None,
                                        op0=AluOp.is_ge)
            selT_ps = o3.tile([E, 256], f32, tag="selT_ps")
            for tt in range(2):
                mkt_ps = o3.tile([28, P], f32r, tag="mkt_ps")
                nc.tensor.transpose(mkt_ps[:], mask_sb[:, tt, :], ident_r_sb[:])
                mkt = ow.tile([28, P], f32r, tag="mkt")
                nc.vector.tensor_copy(mkt[:], mkt_ps[:])
                nc.tensor.matmul(selT_ps[:, tt * 128:(tt + 1) * 128], selmat_sb[:],
                                 mkt[:], start=True, stop=True)
            rwsel_sb = ow.tile([E, 256], f32r, tag="rwsel")
            nc.vector.tensor_mul(rwsel_sb[:], rwT_sb[:], selT_ps[:])
            nrm_ps = o3.tile([1, 256], f32, tag="nrm_ps")
            nc.tensor.matmul(nrm_ps[:], onescol_r_sb[:E, :], rwsel_sb[:],
                             start=True, stop=True)
            nrmr = ow.tile([1, 256], f32r, tag="nrmr")
            nc.vector.reciprocal(nrmr[:], nrm_ps[:])
            nbc_ps = o3.tile([E, 256], f32, tag="nbc_ps")
            nc.tensor.matmul(nbc_ps[:], ones1_r_sb[:1, :E], nrmr[:], start=True, stop=True)
            rts_sb = ow.tile([8, 256], f32, tag="rts_sb")
            nc.vector.tensor_copy(rts_sb[:], selT_ps[:])
            rtw_sb = ow.tile([8, 256], f32, tag="rtw_sb")
            nc.vector.tensor_mul(rtw_sb[:], rwsel_sb[:], nbc_ps[:])
            nc.sync.dma_start(rt_in[0:8, :], rts_sb[:])
            nc.sync.dma_start(rt_in[8:16, :], rtw_sb[:])
            if dbg:
                nc.sync.dma_start(dbg["d_rt"][0:8, :], rts_sb[:])
                nc.sync.dma_start(dbg["d_rt"][8:16, :], rtw_sb[:])

            o3_cm.__exit__(None, None, None)
            o4_cm = tc.tile_pool(name=r + "o4", bufs=2, space="PSUM"); o4 = o4_cm.__enter__()
            # hoisted: h2 -> natural layout for the final residual add (fills
            # PE idle time during the x2 AllGather)
            for tt in range(2):
                for kt in range(8):
                    ht_ps = o4.tile([P, P], f32, tag="ht_ps")
                    nc.tensor.transpose(ht_ps[:], h2_sb[:, kt, tt * 128:(tt + 1) * 128],
                                        ident_sb[:])
                    nc.vector.tensor_copy(fin_nat[:, tt, kt * 128:(kt + 1) * 128],
                                          ht_ps[:])
            # x2 natural bf16
            x2n_sb = opool.tile([P, 2, H], f8)
            for tt in range(2):
                for kt in range(8):
                    xt_ps = o4.tile([P, P], f32r, tag="xt_ps")
                    nc.tensor.transpose(xt_ps[:], x2_sb[:, kt, tt * 128:(tt + 1) * 128],
                                        ident_r_sb[:])
                    nc.vector.tensor_copy(x2n_sb[:, tt, kt * 128:(kt + 1) * 128], xt_ps[:])
            nc.sync.dma_start(xa_in[:].rearrange("(a p) n -> p a n", p=P), x2n_sb[:])
            o4_cm.__exit__(None, None, None)

        if "coll" in skip:
            for cc_ in range(8):
                nc.sync.dma_start(xa_all[cc_ * 256:(cc_ + 1) * 256, :], xa_in[:])
                nc.sync.dma_start(rt_all[cc_ * 16:(cc_ + 1) * 16, :], rt_in[:])
        else:
            nc.gpsimd.collective_compute(
                "AllGather", AluOp.bypass, replica_groups=rg,
                ins=[rt_in.opt()], outs=[rt_all.opt()])
            nc.gpsimd.collective_compute(
                "AllGather", AluOp.bypass, replica_groups=rg,
                ins=[xa_in.opt()], outs=[xa_all.opt()])

        # ================= MoE scope =================
        with tc.tile_pool(name=r + "moe", bufs=1) as mpool, \
             tc.tile_pool(name=r + "mw", bufs=2) as mw:

            # routing rows -> positions -> slot index/weight rows via
            # compare+matmul (PE/DVE work that overlaps the AllGather; the
            # gpsimd queue only carries collectives + the actual gathers)
            mi_cm = tc.tile_pool(name=r + "mi", bufs=1); mi = mi_cm.__enter__()
            sel8_3 = mi.tile([E, NCORES, 256], f32r, tag="selslot")
            nc.sync.dma_start(sel8_3[:],
                              rt_all[:].bitcast(f32r).rearrange("(r x) n -> x r n", x=16)[0:8, :, :])
            wm8_3 = mi.tile([E, NCORES, 256], f32r)
            nc.sync.dma_start(wm8_3[:],
                              rt_all[:].bitcast(f32r).rearrange("(r x) n -> x r n", x=16)[8:16, :, :])
            sel8 = sel8_3[:].rearrange("e r n -> e (r n)")
            wm8 = wm8_3[:].rearrange("e r n -> e (r n)")
            pos8 = mi.tile([E, S], f32, tag="posslot")
            nc.vector.tensor_tensor_scan(pos8[:], sel8, sel8, 0.0,
                                         op0=AluOp.add, op1=AluOp.bypass)
            nc.vector.tensor_scalar(pos8[:], pos8[:], -1.0 - BIGIDX, None, op0=AluOp.add)
            posm8 = mi.tile([E, S], f32r)
            nc.vector.tensor_mul(posm8[:], pos8[:], sel8)
            nc.vector.tensor_scalar(posm8[:], posm8[:], BIGIDX, None, op0=AluOp.add)
            m1c_cm = tc.tile_pool(name=r + "m1c", bufs=1, space="PSUM"); m1c = m1c_cm.__enter__()
            posmy = mi.tile([1, S], f32, tag="posslot")
            wmmy = mi.tile([1, S], f32, tag="selslot")
            for ntile in range(4):
                nsl = slice(ntile * 512, (ntile + 1) * 512)
                pp = m1c.tile([1, 512], f32, tag="pp")
                nc.tensor.matmul(pp[:], onehot_r_sb[:], posm8[:, nsl],
                                 start=True, stop=True)
                nc.vector.tensor_copy(posmy[:, nsl], pp[:])
                wp = m1c.tile([1, 512], f32, tag="wp")
                nc.tensor.matmul(wp[:], onehot_r_sb[:], wm8[:, nsl],
                                 start=True, stop=True)
                nc.vector.tensor_copy(wmmy[:, nsl], wp[:])
            m1c_cm.__exit__(None, None, None)
            # accumulate idxval / wm / occupancy slot rows over all token tiles
            m1b_cm = tc.tile_pool(name=r + "m1b", bufs=1, space="PSUM"); m1b = m1b_cm.__enter__()
            m1_cm = tc.tile_pool(name=r + "m1", bufs=1, space="PSUM"); m1 = m1_cm.__enter__()
            accI = []
            accW = []
            accO = []
            for ch in range(2):
                aI = m1.tile([1, 320], f32, tag=f"accI{ch}")
                aW = m1.tile([1, 320], f32, tag=f"accW{ch}")
                aO = m1.tile([1, 320], f32, tag=f"accO{ch}")
                accI.append(aI); accW.append(aW); accO.append(aO)
            for ti in ([] if "front" in skip else range(NT)):
                po_ps = m1b.tile([P, 1], f32, tag="po_ps")
                nc.tensor.transpose(po_ps[:], posmy[:1, ti * 128:(ti + 1) * 128],
                                    ident_sb[:1, :1])
                wm_ps = m1b.tile([P, 1], f32, tag="wm_ps")
                nc.tensor.transpose(wm_ps[:], wmmy[:1, ti * 128:(ti + 1) * 128],
                                    ident_sb[:1, :1])
                stg = mw.tile([P, 3], f32r, tag="stg")
                nc.vector.tensor_copy(stg[:], iota3_sb[:, ti, :])
                nc.vector.tensor_copy(stg[:, 1:2], wm_ps[:])
                po_col = mw.tile([P, 1], f32, tag="po_col")
                nc.vector.tensor_copy(po_col[:], po_ps[:])
                M_ti = mw.tile([P, CAP], f32r, tag="M_ti")
                nc.vector.tensor_scalar(M_ti[:], iota640_sb[:], po_col[:], None,
                                        op0=AluOp.is_equal)
                st_, sp_ = ti == 0, ti == NT - 1
                for ch in range(2):
                    msl = slice(ch * 320, (ch + 1) * 320)
                    nc.tensor.matmul(accI[ch][:], stg[:, 0:1], M_ti[:, msl],
                                     start=st_, stop=sp_)
                    nc.tensor.matmul(accW[ch][:], stg[:, 1:2], M_ti[:, msl],
                                     start=st_, stop=sp_)
                    nc.tensor.matmul(accO[ch][:], stg[:, 2:3], M_ti[:, msl],
                                     start=st_, stop=sp_)
            idxrow = mpool.tile([1, CAP], f32)
            wmrow = mpool.tile([1, CAP], f32)
            for ch in range(2):
                msl = slice(ch * 320, (ch + 1) * 320)
                nofix = mw.tile([1, 320], f32, tag="nofix")
                nc.vector.tensor_scalar(nofix[:], accO[ch][:], -HUGE, HUGE,
                                        op0=AluOp.mult, op1=AluOp.add)
                nc.vector.tensor_add(idxrow[:, msl], accI[ch][:], nofix[:])
                nc.vector.tensor_copy(wmrow[:, msl], accW[ch][:])
            m1_cm.__exit__(None, None, None)
            m1b_cm.__exit__(None, None, None)
            mi_cm.__exit__(None, None, None)
            m2_cm = tc.tile_pool(name=r + "m2", bufs=2, space="PSUM"); m2 = m2_cm.__enter__()
            # idx/wm columns + gather + transpose
            xg_sb = mpool.tile([P, 8, CAP], f8)
            wmg_sb = mpool.tile([P, NCAP], f32)
            idx_tiles = []
            for j in range(NCAP):
                ip_ps = m2.tile([P, 1], f32, tag="ip_ps")
                nc.tensor.transpose(ip_ps[:], idxrow[:1, j * 128:(j + 1) * 128],
                                    ident_sb[:1, :1])
                idxj = mpool.tile([P, 1], i32, tag=f"idxj{j}")
                idx_tiles.append(idxj)
                nc.vector.tensor_copy(idxj[:], ip_ps[:])
                wp_ps = m2.tile([P, 1], f32, tag="wp_ps")
                nc.tensor.transpose(wp_ps[:], wmrow[:1, j * 128:(j + 1) * 128],
                                    ident_sb[:1, :1])
                nc.vector.tensor_copy(wmg_sb[:, j:j + 1], wp_ps[:])
                if "front" in skip:
                    continue
                gat = mw.tile([P, H], f8, tag="gat")
                nc.gpsimd.indirect_dma_start(
                    out=gat[:], out_offset=None, in_=xa_all[:],
                    in_offset=bass.IndirectOffsetOnAxis(ap=idxj[:, :1], axis=0),
                    bounds_check=S - 1, oob_is_err=False)
                for kt in range(8):
                    gt_ps = m2.tile([P, 2 * P], f8, tag="gt_ps")
                    nc.tensor.transpose(gt_ps[:, 0:256:2], gat[:, kt * 128:(kt + 1) * 128],
                                        ident_f8_sb[:])
                    nc.scalar.activation(xg_sb[:, kt, j * 128:(j + 1) * 128],
                                         gt_ps[:, 0:256:2], Act.Copy)
            if dbg:
                didx = mw.tile([P, NCAP], i32, tag="didx")
                for j in range(NCAP):
                    nc.vector.tensor_copy(didx[:, j:j + 1], idx_tiles[j][:])
                nc.sync.dma_start(dbg["d_idx"], didx[:])
                nc.gpsimd.dma_start(dbg["d_xg"], xg_sb[:, 0, :])

            m2_cm.__exit__(None, None, None)
            msil_cm = tc.tile_pool(name=r + "msil", bufs=3); msil = msil_cm.__enter__()
            m3_cm = tc.tile_pool(name=r + "m3", bufs=4, space="PSUM"); m3 = m3_cm.__enter__()
            wpool_cm = tc.tile_pool(name=r + "wpre", bufs=6); wpre = wpool_cm.__enter__()
            # FFN phase A
            hp_sb = mpool.tile([P, NI, CAP], f8)
            for it in ([] if "ffn" in skip else range(NI)):
                w13_sb = wpre.tile([P, 8, 256], f8, tag="w13_sb")
                nc.sync.dma_start(w13_sb[:], t["w13"][it].rearrange("(kt p) n -> p kt n", p=P))
                for hf in range(2):
                    csl = slice(hf * 320, hf * 320 + 320)
                    h1p = m3.tile([P, 320], f32, tag="h1")
                    h3p = m3.tile([P, 320], f32, tag="h3")
                    for kp in range(4):
                        ksl = slice(2 * kp, 2 * kp + 2)
                        st, sp = kp == 0, kp == 3
                        nc.tensor.matmul(h1p[:], w13_sb[:, ksl, 0:128], xg_sb[:, ksl, csl],
                                         start=st, stop=sp, perf_mode=DR)
                        nc.tensor.matmul(h3p[:], w13_sb[:, ksl, 128:256], xg_sb[:, ksl, csl],
                                         start=st, stop=sp, perf_mode=DR)
                    sg = msil.tile([P, 320], f32, tag="sg")
                    nc.scalar.activation(sg[:], h1p[:], Act.Sigmoid, scale=1.0 / W13_SCALE)
                    hp1 = msil.tile([P, 320], f32, tag="hp1")
                    nc.vector.tensor_mul(hp1[:], h1p[:], sg[:])
                    hpf = msil.tile([P, 320], f32, tag="hpf")
                    nc.vector.tensor_mul(hpf[:], hp1[:], h3p[:])
                    nc.scalar.activation(hp_sb[:, it, csl], hpf[:], Act.Copy,
                                         scale=HP_SCALE / (W13_SCALE * W13_SCALE))
            if dbg:
                nc.gpsimd.dma_start(dbg["d_hp"], hp_sb[:, 0, :])

            wpool_cm.__exit__(None, None, None)
            m3_cm.__exit__(None, None, None)
            msil_cm.__exit__(None, None, None)
            # FFN phase B, computed transposed: out[tok, h] = sum_i hp[i,tok]*w2[i,h]
            # (hp chunks as weights, w2 rows streamed; no output transposes)
            out_nat = mpool.tile([P, NCAP, H], bf16)
            if "ffn" not in skip:
                m4_cm = tc.tile_pool(name=r + "m4", bufs=1, space="PSUM")
                m4 = m4_cm.__enter__()
                for hh in range(2):
                    hsl = slice(hh * 512, (hh + 1) * 512)
                    mo_ps = []
                    for j in range(NCAP):
                        mo_j = m4.tile([P, 512], f32, tag=f"mo{j}")
                        mo_ps.append(mo_j)
                    for ip in range(NI // 2):
                        w2t = mw.tile([P, 2, 512], f8, tag="w2t")
                        nc.sync.dma_start(
                            w2t[:],
                            t["w2s"][2 * ip:2 * ip + 2, :, hsl].rearrange("k p n -> p k n"))
                        for j in range(NCAP):
                            nc.tensor.matmul(mo_ps[j][:],
                                             hp_sb[:, 2 * ip:2 * ip + 2, j * 128:(j + 1) * 128],
                                             w2t[:], start=(ip == 0), stop=(ip == NI // 2 - 1),
                                             perf_mode=DR)
                    for j in range(NCAP):
                        nc.vector.tensor_scalar(out_nat[:, j, hsl], mo_ps[j][:],
                                                wmg_sb[:, j:j + 1],
                                                1.0 / (HP_SCALE * W2_SCALE),
                                                op0=AluOp.mult, op1=AluOp.mult)
                m4_cm.__exit__(None, None, None)
            for j in ([] if "ffn" in skip else range(NCAP)):
                nc.gpsimd.indirect_dma_start(
                    out=moe_acc[:],
                    out_offset=bass.IndirectOffsetOnAxis(ap=idx_tiles[j][:, :1], axis=0),
                    in_=out_nat[:, j, :], in_offset=None,
                    bounds_check=S - 1, oob_is_err=False)

        if "coll" in skip:
            nc.sync.dma_start(rs_out[:], moe_acc[0:256, :])
        else:
            nc.gpsimd.collective_compute(
                "ReduceScatter", AluOp.add, replica_groups=rg,
                ins=[moe_acc.opt()], outs=[rs_out.opt()])

        # ================= final =================
        with tc.tile_pool(name=r + "fin", bufs=2) as fw:
            rs_sb = fw.tile([P, 2, H], bf16, tag="rs_sb")
            nc.sync.dma_start(rs_sb[:], rs_out[:].rearrange("(a p) n -> p a n", p=P))
            if dbg:
                nc.gpsimd.dma_start(dbg["d_moe"].rearrange("(a p) n -> p a n", p=P),
                                    rs_sb[:])
            fin_sb = fw.tile([P, 2, H], f32, tag="fin_sb")
            for tt in range(2):
                nc.vector.tensor_add(fin_sb[:, tt, :], fin_nat[:, tt, :],
                                     rs_sb[:, tt, :])
            nc.sync.dma_start(t["out_blk"].rearrange("(a p) n -> p a n", p=P), fin_sb[:])


# ======================= host side =======================

def _rope_tables():
    pos = np.arange(S, dtype=np.float64)
    inv = 1.0 / (THETA ** (np.arange(0, HD, 2, dtype=np.float64) / HD))
    fr = pos[:, None] * inv[None, :]
    emb = np.concatenate([fr, fr], axis=-1)
    return np.cos(emb).astype(np.float32), np.sin(emb).astype(np.float32)


def _prep_inputs(inputs):
    hs = np.asarray(inputs["hidden_states"], np.float32)[0]
    ln1 = np.asarray(inputs["ln1_w"], np.float32)
    ln2 = np.asarray(inputs["ln2_w"], np.float32)
    wq = np.asarray(inputs["wq"], np.float32) * ln1[None, :] / np.sqrt(HD)
    wk = np.asarray(inputs["wk"], np.float32) * ln1[None, :]
    wv = np.asarray(inputs["wv"], np.float32) * ln1[None, :]
    wo = np.asarray(inputs["wo"], np.float32)
    gate = np.asarray(inputs["gate_w"], np.float32) * ln2[None, :]
    w1 = np.asarray(inputs["w1"], np.float32) * ln2[None, None, :]
    w3 = np.asarray(inputs["w3"], np.float32) * ln2[None, None, :]
    w2 = np.asarray(inputs["w2"], np.float32)

    cos, sin = _rope_tables()
    hT = np.ascontiguousarray(hs.T)

    def rot_w(w_head):  # [64, H] -> R @ w: rows = rotate_half structure
        return np.concatenate([-w_head[32:64], w_head[0:32]], axis=0)

    ident = np.eye(P, dtype=np.float32)
    sidx = np.arange(P)
    tri_m = np.where(sidx[:, None] <= sidx[None, :], 0.0, NEG).astype(np.float32)
    onescol = np.ones((P, 1), np.float32)
    EPS_ = EPS
    ones1 = np.ones((1, P), np.float32)
    mcomb = np.zeros((E, 28), np.float32)
    for ci, (a, b) in enumerate(COMBS):
        mcomb[a, ci] = 1.0
        mcomb[b, ci] = 1.0
    selmat = np.ascontiguousarray(mcomb.T)
    iota3 = np.zeros((P, NT, 3), np.float32)
    for ti in range(NT):
        z, a = ti // 2, ti % 2
        iota3[:, ti, 0] = z * 256 + a * P + np.arange(P)
    iota3[:, :, 2] = 1.0
    iota640 = np.tile(np.arange(CAP, dtype=np.float32)[None, :], (P, 1))
    zrow = np.zeros((P, 4096), BF16_NP)
    cosT_d = np.ascontiguousarray(
        np.concatenate([cos.T, cos.T], axis=0))  # [128, S]
    sinT_d = np.ascontiguousarray(np.concatenate([sin.T, sin.T], axis=0))

    in_maps = []
    for c in range(NCORES):
        bA, bB = c, 15 - c
        tok = np.concatenate([np.arange(bA * P, bA * P + P),
                              np.arange(bB * P, bB * P + P)])
        qh0, qh1, kvh = 2 * c, 2 * c + 1, c
        wq0 = wq[qh0 * HD:(qh0 + 1) * HD]
        wq1 = wq[qh1 * HD:(qh1 + 1) * HD]
        wkc = wk[kvh * HD:(kvh + 1) * HD]
        wvc = wv[kvh * HD:(kvh + 1) * HD]
        wqkv = np.concatenate([
            wq0.T, wq1.T, rot_w(wq0).T, rot_w(wq1).T,
            wkc.T, rot_w(wkc).T, wvc.T], axis=1).astype(np.float32)
        onehot = np.zeros((E, 1), np.float32)
        onehot[c, 0] = 1.0
        m = {
            "hT_my": np.ascontiguousarray(hT[:, tok]),
            "hT_full": hT,
            "wqkvT": np.ascontiguousarray(wqkv),
            "woT": wo.T.astype(BF16_NP),
            "gateT": np.ascontiguousarray(gate.T),
            "cosT": cosT_d, "sinT": sinT_d,
            "ident": ident, "tri": tri_m,
            "onescol": onescol, "onescol_r": onescol, "ones1_r": ones1,
            "epscol": np.full((P, 1), EPS, np.float32),
            "onehot_r": onehot, "mcomb_r": mcomb, "selmat_r": selmat,
            "iota3": iota3, "iota640": iota640, "zrow": zrow,
            "w13": (np.ascontiguousarray(np.concatenate(
                [w1[c].reshape(NI, P, H).transpose(0, 2, 1),
                 w3[c].reshape(NI, P, H).transpose(0, 2, 1)],
                axis=2)) * W13_SCALE).astype(F8_NP),
            "w2s": (np.ascontiguousarray(w2[c].T.reshape(NI, P, H)) * W2_SCALE).astype(F8_NP),
        }
        in_maps.append(m)
    return in_maps


_CACHE = {}


def _get_program(dbg=False, n_unroll=1):
    key = (dbg, n_unroll)
    if key not in _CACHE:
        _CACHE[key] = build_program(dbg=dbg, n_unroll=n_unroll)
    return _CACHE[key]


def run(inputs, dbg=False, n_unroll=1):
    nc = _get_program(dbg=dbg, n_unroll=n_unroll)
    in_maps = _prep_inputs(inputs)
    return bass_utils.run_bass_kernel_spmd(nc, in_maps, core_ids=list(range(NCORES)))


def assemble_output(res):
    out = np.zeros((S, H), np.float32)
    for c in range(NCORES):
        blk = res.results[c]["out_blk"]
        bA, bB = c, 15 - c
        out[bA * P:(bA + 1) * P] = blk[0:P]
        out[bB * P:(bB + 1) * P] = blk[P:256]
    return out.reshape(B, S, H)


def kernel(**inputs):
    res = run(inputs)
    return assemble_output(res)

